# revision 8
# baseline (speedup 1.0000x reference)
"""Trainium2 Bass kernel for nn_NeuroNet_231928234454 (retrieval_knn).

Computation (reference):
    mask = (inputs != 0)                               [S]
    x    = (embed_w[:,None,:] + rel_w[None,:,:]) * mask  -> [S*REL, D]
    kg   = (dise_w @ x.T) / max(|dise_w| |x|, eps)       [DD, S*REL]  (cosine)
    hidden = concat([rule_features, kg], axis=1).reshape(1, -1)
    out  = sigmoid(hidden @ fc_w.T + fc_b)               [1, DD]

The 545 MB fc_w stream dominates (memory-bound). Sharding: core c owns
disease-rows 8c..8c+8 of the hidden matrix [DD, 33280] (column-shard of
fc_w's 2.1M input dim into 8 contiguous blocks). Each core computes its
8 rows of kg on-chip from tiny inputs, then a PE mat-vec against its
fc_w slice (host re-laid-out so every DMA is contiguous and every
matmul's stationary operand packs two 128-element hidden chunks x 64
outputs). Partial [128,2] psum results are folded/summed on host with
fc_b + sigmoid (64 numbers).

Host precomputes (exactly, in f64): row-norm reciprocals of dise_w,
binv[n] = mask[s]/||embed_s + rel_r||, and B = (dise_w/dn) @ rel_w.T so
the device never needs the masked x tensor:
    kg[k, n] = (A[k, s] + B[k, r]) * binv[n],  A = (dise_w/dn) @ embed_w.T
"""

import sys

if "/opt/trn_rl_repo" not in sys.path:
    sys.path.insert(0, "/opt/trn_rl_repo")

import numpy as np

import concourse.bacc as bacc
import concourse.mybir as mybir
from concourse.bass_utils import run_bass_kernel_spmd
from concourse.tile import TileContext

# -- problem constants (hardcoded; harness provides full-shape inputs) --
S, D, DD, REL, R = 4096, 128, 64, 8, 512
N_CORES = 8
KK = DD // N_CORES          # 8 disease rows per core
SLICE = REL * S + R         # 33280 hidden elems per disease row
L = KK * SLICE              # 266240 hidden elems per core
NCHUNK = L // 128           # 2080 chunks of 128
NPAIR = NCHUNK // 2         # 1040 fc matmuls (2 chunks per stationary)
DMA_GROUP = 16              # fc matmuls per W DMA tile
NDMA = NPAIR // DMA_GROUP   # 65 DMA tiles of [128, 2048]
S2 = S // 128               # 32 chunks along S
RQ = R // 128               # 4 chunks of rule features per disease

# fc_w streaming dtype. float32 is exact; float16 halves HBM traffic
# (the dominant cost) at ~1e-3 relative output error.
W_DTYPE = mybir.dt.float16
W_NP = np.float16
W_BUFS = 24 if W_NP == np.float16 else 14

LAST_RESULTS = None  # BassKernelResults of the most recent run (for test.py)
_CACHE = {}


def _build_bass():
    # Bacc (not raw Bass): its finalize() splits multi-sem waits into
    # event semaphores, which walrus codegen requires (max 1 wait/inst).
    nc = bacc.Bacc("TRN2", target_bir_lowering=False, debug=False)
    f32 = mybir.dt.float32
    wdt = W_DTYPE

    wt = nc.dram_tensor(
        "wt", [NDMA, 128, DMA_GROUP * 128], wdt, kind="ExternalInput"
    )
    # All small f32 constants in ONE tensor -> one DMA -> one semaphore
    # (walrus allows only a single sync wait on a matmul's LDWEIGHTS).
    # cols: [0:S] embedT | [S:S+KK] diseT | [+KK*REL] brep | [+REL*S2] binvT
    CONST_COLS = S + KK + KK * REL + REL * S2
    consts = nc.dram_tensor("consts", [128, CONST_COLS], f32, kind="ExternalInput")
    rfT = nc.dram_tensor("rfT", [128, KK * RQ], wdt, kind="ExternalInput")
    partials = nc.dram_tensor("partials", [128, 2], f32, kind="ExternalOutput")

    with TileContext(nc) as tc:
        with (
            tc.tile_pool(name="const", bufs=1) as cpool,
            tc.tile_pool(name="wpool", bufs=W_BUFS) as wpool,
            tc.tile_pool(name="psum", bufs=1, space="PSUM") as ppool,
        ):
            const_sb = cpool.tile([128, CONST_COLS], f32)
            nc.sync.dma_start(out=const_sb[:], in_=consts[:])
            emb_sb = const_sb[:, 0:S]
            dise_sb = const_sb[:, S : S + KK]
            brep_sb = const_sb[:, S + KK : S + KK + KK * REL]
            binv_sb = const_sb[:, S + KK + KK * REL : CONST_COLS]

            # hidden vector, chunk-column layout: hid[p, c] = hidden chunk c
            # elem p. c < 2048: (kk, r, s2) = kg value for s = s2*128+p.
            # c >= 2048: (kk, q) = rule_features[kk, q*128+p].
            hid = cpool.tile([128, NCHUNK], wdt)
            nc.sync.dma_start(out=hid[:, KK * 256 : NCHUNK], in_=rfT[:])

            # A[p, kk, s2] = sum_e embedT[e, s2*128+p] * diseT[e, kk]
            psumA = ppool.tile([128, KK, S2], f32)
            for s2 in range(S2):
                nc.tensor.matmul(
                    out=psumA[:, :, s2],
                    lhsT=emb_sb[:, s2 * 128 : (s2 + 1) * 128],
                    rhs=dise_sb,
                    start=True,
                    stop=True,
                )

            # kg -> hid: (A + B[kk,r]) * binv
            for kk in range(KK):
                for r in range(REL):
                    nc.vector.scalar_tensor_tensor(
                        out=hid[:, kk * 256 + r * S2 : kk * 256 + (r + 1) * S2],
                        in0=psumA[:, kk, :],
                        scalar=brep_sb[:, kk * REL + r : kk * REL + r + 1],
                        in1=binv_sb[:, r * S2 : (r + 1) * S2],
                        op0=mybir.AluOpType.add,
                        op1=mybir.AluOpType.mult,
                    )

            # fc mat-vec: each stationary packs chunks (2t, 2t+1) as
            # [128, sub*64+d]; rhs = hid[:, 2t:2t+2]. psumF[d,0] and
            # psumF[64+d,1] accumulate the two true partial dots.
            psumF = ppool.tile([128, 2], f32)
            for t2 in range(NDMA):
                wtile = wpool.tile([128, DMA_GROUP * 128], wdt)
                nc.sync.dma_start(out=wtile[:], in_=wt[t2, :, :])
                for i in range(DMA_GROUP):
                    t = t2 * DMA_GROUP + i
                    nc.tensor.matmul(
                        out=psumF[:, :],
                        lhsT=wtile[:, i * 128 : (i + 1) * 128],
                        rhs=hid[:, 2 * t : 2 * t + 2],
                        start=(t == 0),
                        stop=(t == NPAIR - 1),
                    )

            out_sb = cpool.tile([128, 2], f32)
            nc.vector.tensor_copy(out=out_sb[:], in_=psumF[:, :])
            nc.sync.dma_start(out=partials[:], in_=out_sb[:])

    # Run Bacc's compile passes (register allocation, wait splitting into
    # event semaphores) — nothing else in the axon/PJRT path does this.
    nc.finalize()
    return nc


def get_nc():
    if "nc" not in _CACHE:
        _CACHE["nc"] = _build_bass()
    return _CACHE["nc"]


def make_in_maps(inputs, rule_features, embed_w, rel_w, dise_w, fc_w):
    """Host-side sharding + relayout of the full inputs into per-core maps."""
    mask = np.asarray(inputs) != 0  # [S]
    embed64 = np.asarray(embed_w, dtype=np.float64)
    rel64 = np.asarray(rel_w, dtype=np.float64)
    dise64 = np.asarray(dise_w, dtype=np.float64)

    # binv[s, r] = mask[s] / ||embed_s + rel_r||
    sq = (
        (embed64**2).sum(1)[:, None]
        + 2.0 * (embed64 @ rel64.T)
        + (rel64**2).sum(1)[None, :]
    )  # [S, REL]
    binv = np.where(mask[:, None], 1.0 / np.sqrt(np.maximum(sq, 1e-30)), 0.0)
    # [p, r*32+s2] with s = s2*128+p
    binv_t = np.ascontiguousarray(
        binv.reshape(S2, 128, REL).transpose(1, 2, 0).reshape(128, REL * S2),
        dtype=np.float32,
    )

    dn = np.sqrt((dise64**2).sum(1))  # [DD]
    dise_sc = dise64 / dn[:, None]  # [DD, D]
    B = dise_sc @ rel64.T  # [DD, REL]

    embedT = np.ascontiguousarray(np.asarray(embed_w).T, dtype=np.float32)

    # [kslot, d, 33280], contiguous rows for cache-friendly chunk slicing
    fc3 = np.asarray(fc_w).reshape(DD, DD, SLICE)
    acp = np.ascontiguousarray(fc3.transpose(1, 0, 2))

    rf = np.asarray(rule_features)

    in_maps = []
    for c in range(N_CORES):
        k0 = KK * c
        consts_c = np.empty((128, S + KK + KK * REL + REL * S2), dtype=np.float32)
        consts_c[:, 0:S] = embedT
        consts_c[:, S : S + KK] = dise_sc[k0 : k0 + KK].T
        consts_c[:, S + KK : S + KK + KK * REL] = B[k0 : k0 + KK].reshape(
            1, KK * REL
        )
        consts_c[:, S + KK + KK * REL :] = binv_t
        rfT_c = np.ascontiguousarray(
            rf[k0 : k0 + KK].reshape(KK, RQ, 128).transpose(2, 0, 1).reshape(128, KK * RQ),
            dtype=W_NP,
        )

        w_c = np.empty((NDMA, 128, DMA_GROUP * 128), dtype=W_NP)
        wv = w_c.reshape(NDMA, 128, DMA_GROUP, 2, 64)  # [t2, k, i, sub, d]
        block = acp[k0 : k0 + KK]  # [kk, d, SLICE]
        for kk in range(KK):
            for s2 in range(S2):
                src = block[kk][:, R + s2 * 1024 : R + (s2 + 1) * 1024]
                chunkT = np.ascontiguousarray(
                    src.reshape(DD, 128, REL).transpose(2, 1, 0), dtype=W_NP
                )  # [r, p, d]
                for r in range(REL):
                    cc = kk * 256 + r * S2 + s2
                    t, sub = cc >> 1, cc & 1
                    wv[t >> 4, :, t & 15, sub, :] = chunkT[r]
            for q in range(RQ):
                cc = KK * 256 + kk * RQ + q
                t, sub = cc >> 1, cc & 1
                wv[t >> 4, :, t & 15, sub, :] = np.ascontiguousarray(
                    block[kk][:, q * 128 : (q + 1) * 128].T, dtype=W_NP
                )

        in_maps.append({"wt": w_c, "consts": consts_c, "rfT": rfT_c})
    return in_maps


def finish(partials_list, fc_b):
    """Fold per-core [128, 2] psum outputs -> sigmoid(logits) [1, DD]."""
    acc = np.zeros(DD, dtype=np.float64)
    for p in partials_list:
        p = np.asarray(p, dtype=np.float64)
        acc += p[:DD, 0] + p[DD:, 1]
    logits = acc + np.asarray(fc_b, dtype=np.float64)
    out = 1.0 / (1.0 + np.exp(-logits))
    return out.reshape(1, DD).astype(np.float32)


def kernel(inputs, rule_features, embed_w, rel_w, dise_w, fc_w, fc_b):
    global LAST_RESULTS
    nc = get_nc()
    in_maps = make_in_maps(inputs, rule_features, embed_w, rel_w, dise_w, fc_w)
    LAST_RESULTS = run_bass_kernel_spmd(nc, in_maps, core_ids=list(range(N_CORES)))
    return finish([m["partials"] for m in LAST_RESULTS.results], fc_b)


# revision 11
# speedup vs baseline: 1.0131x; 1.0131x over previous
"""Trainium2 Bass kernel for nn_NeuroNet_231928234454 (retrieval_knn).

Computation (reference):
    mask = (inputs != 0)                               [S]
    x    = (embed_w[:,None,:] + rel_w[None,:,:]) * mask  -> [S*REL, D]
    kg   = (dise_w @ x.T) / max(|dise_w| |x|, eps)       [DD, S*REL]  (cosine)
    hidden = concat([rule_features, kg], axis=1).reshape(1, -1)
    out  = sigmoid(hidden @ fc_w.T + fc_b)               [1, DD]

The 545 MB fc_w stream dominates (memory-bound). Sharding: core c owns
disease-rows 8c..8c+8 of the hidden matrix [DD, 33280] (column-shard of
fc_w's 2.1M input dim into 8 contiguous blocks). Each core computes its
8 rows of kg on-chip from tiny inputs, then a PE mat-vec against its
fc_w slice (host re-laid-out so every DMA is contiguous and every
matmul's stationary operand packs two 128-element hidden chunks x 64
outputs). Partial [128,2] psum results are folded/summed on host with
fc_b + sigmoid (64 numbers).

Host precomputes (exactly, in f64): row-norm reciprocals of dise_w,
binv[n] = mask[s]/||embed_s + rel_r||, and B = (dise_w/dn) @ rel_w.T so
the device never needs the masked x tensor:
    kg[k, n] = (A[k, s] + B[k, r]) * binv[n],  A = (dise_w/dn) @ embed_w.T
"""

import sys

if "/opt/trn_rl_repo" not in sys.path:
    sys.path.insert(0, "/opt/trn_rl_repo")

import numpy as np

import concourse.bacc as bacc
import concourse.mybir as mybir
from concourse.bass_utils import run_bass_kernel_spmd
from concourse.tile import TileContext

# -- problem constants (hardcoded; harness provides full-shape inputs) --
S, D, DD, REL, R = 4096, 128, 64, 8, 512
N_CORES = 8
KK = DD // N_CORES          # 8 disease rows per core
SLICE = REL * S + R         # 33280 hidden elems per disease row
L = KK * SLICE              # 266240 hidden elems per core
NCHUNK = L // 128           # 2080 chunks of 128
NPAIR = NCHUNK // 2         # 1040 fc matmuls (2 chunks per stationary)
DMA_GROUP = 16              # fc matmuls per W DMA tile
NDMA = NPAIR // DMA_GROUP   # 65 DMA tiles of [128, 2048]
S2 = S // 128               # 32 chunks along S
RQ = R // 128               # 4 chunks of rule features per disease

# fc_w streaming dtype. float32 is exact; float16 halves HBM traffic
# (the dominant cost) at ~1e-3 relative output error.
W_DTYPE = mybir.dt.float16
W_NP = np.float16
W_BUFS = 24 if W_NP == np.float16 else 14

LAST_RESULTS = None  # BassKernelResults of the most recent run (for test.py)
_CACHE = {}


def _build_bass():
    # Bacc (not raw Bass): its finalize() splits multi-sem waits into
    # event semaphores, which walrus codegen requires (max 1 wait/inst).
    nc = bacc.Bacc("TRN2", target_bir_lowering=False, debug=False)
    f32 = mybir.dt.float32
    wdt = W_DTYPE

    wt = nc.dram_tensor(
        "wt", [NDMA, 128, DMA_GROUP * 128], wdt, kind="ExternalInput"
    )
    # Small constants packed into ONE tensor per dtype -> one DMA -> one
    # semaphore each (walrus allows only a single sync wait on a matmul's
    # LDWEIGHTS). fp16: [0:S] embedT | [S:S+KK] diseT (feeds the A-matmul,
    # halves its DMA bytes). f32: [0:KK*REL] brep | [KK*REL:] binvT.
    C16_COLS = S + KK
    C32_COLS = KK * REL + REL * S2
    consts16 = nc.dram_tensor("consts16", [128, C16_COLS], wdt, kind="ExternalInput")
    consts32 = nc.dram_tensor("consts32", [128, C32_COLS], f32, kind="ExternalInput")
    rfT = nc.dram_tensor("rfT", [128, KK * RQ], wdt, kind="ExternalInput")
    partials = nc.dram_tensor("partials", [128, 2], f32, kind="ExternalOutput")

    with TileContext(nc) as tc:
        with (
            tc.tile_pool(name="const", bufs=1) as cpool,
            tc.tile_pool(name="wpool", bufs=W_BUFS) as wpool,
            tc.tile_pool(name="psum", bufs=1, space="PSUM") as ppool,
        ):
            c16_sb = cpool.tile([128, C16_COLS], wdt)
            nc.sync.dma_start(out=c16_sb[:], in_=consts16[:])
            c32_sb = cpool.tile([128, C32_COLS], f32)
            nc.sync.dma_start(out=c32_sb[:], in_=consts32[:])
            emb_sb = c16_sb[:, 0:S]
            dise_sb = c16_sb[:, S : S + KK]
            brep_sb = c32_sb[:, 0 : KK * REL]
            binv_sb = c32_sb[:, KK * REL : C32_COLS]

            # hidden vector, chunk-column layout: hid[p, c] = hidden chunk c
            # elem p. c < 2048: (kk, r, s2) = kg value for s = s2*128+p.
            # c >= 2048: (kk, q) = rule_features[kk, q*128+p].
            hid = cpool.tile([128, NCHUNK], wdt)
            nc.sync.dma_start(out=hid[:, KK * 256 : NCHUNK], in_=rfT[:])

            # A[p, kk, s2] = sum_e embedT[e, s2*128+p] * diseT[e, kk]
            psumA = ppool.tile([128, KK, S2], f32)
            for s2 in range(S2):
                nc.tensor.matmul(
                    out=psumA[:, :, s2],
                    lhsT=emb_sb[:, s2 * 128 : (s2 + 1) * 128],
                    rhs=dise_sb,
                    start=True,
                    stop=True,
                )

            # kg -> hid: (A + B[kk,r]) * binv
            for kk in range(KK):
                for r in range(REL):
                    nc.vector.scalar_tensor_tensor(
                        out=hid[:, kk * 256 + r * S2 : kk * 256 + (r + 1) * S2],
                        in0=psumA[:, kk, :],
                        scalar=brep_sb[:, kk * REL + r : kk * REL + r + 1],
                        in1=binv_sb[:, r * S2 : (r + 1) * S2],
                        op0=mybir.AluOpType.add,
                        op1=mybir.AluOpType.mult,
                    )

            # fc mat-vec: each stationary packs chunks (2t, 2t+1) as
            # [128, sub*64+d]; rhs = hid[:, 2t:2t+2]. psumF[d,0] and
            # psumF[64+d,1] accumulate the two true partial dots.
            psumF = ppool.tile([128, 2], f32)
            for t2 in range(NDMA):
                wtile = wpool.tile([128, DMA_GROUP * 128], wdt)
                nc.sync.dma_start(out=wtile[:], in_=wt[t2, :, :])
                for i in range(DMA_GROUP):
                    t = t2 * DMA_GROUP + i
                    nc.tensor.matmul(
                        out=psumF[:, :],
                        lhsT=wtile[:, i * 128 : (i + 1) * 128],
                        rhs=hid[:, 2 * t : 2 * t + 2],
                        start=(t == 0),
                        stop=(t == NPAIR - 1),
                    )

            out_sb = cpool.tile([128, 2], f32)
            nc.vector.tensor_copy(out=out_sb[:], in_=psumF[:, :])
            nc.sync.dma_start(out=partials[:], in_=out_sb[:])

    # Run Bacc's compile passes (register allocation, wait splitting into
    # event semaphores) — nothing else in the axon/PJRT path does this.
    nc.finalize()
    return nc


def get_nc():
    if "nc" not in _CACHE:
        _CACHE["nc"] = _build_bass()
    return _CACHE["nc"]


def make_in_maps(inputs, rule_features, embed_w, rel_w, dise_w, fc_w):
    """Host-side sharding + relayout of the full inputs into per-core maps."""
    mask = np.asarray(inputs) != 0  # [S]
    embed64 = np.asarray(embed_w, dtype=np.float64)
    rel64 = np.asarray(rel_w, dtype=np.float64)
    dise64 = np.asarray(dise_w, dtype=np.float64)

    # binv[s, r] = mask[s] / ||embed_s + rel_r||
    sq = (
        (embed64**2).sum(1)[:, None]
        + 2.0 * (embed64 @ rel64.T)
        + (rel64**2).sum(1)[None, :]
    )  # [S, REL]
    binv = np.where(mask[:, None], 1.0 / np.sqrt(np.maximum(sq, 1e-30)), 0.0)
    # [p, r*32+s2] with s = s2*128+p
    binv_t = np.ascontiguousarray(
        binv.reshape(S2, 128, REL).transpose(1, 2, 0).reshape(128, REL * S2),
        dtype=np.float32,
    )

    dn = np.sqrt((dise64**2).sum(1))  # [DD]
    dise_sc = dise64 / dn[:, None]  # [DD, D]
    B = dise_sc @ rel64.T  # [DD, REL]

    embedT = np.ascontiguousarray(np.asarray(embed_w).T, dtype=np.float32)

    # [kslot, d, 33280], contiguous rows for cache-friendly chunk slicing
    fc3 = np.asarray(fc_w).reshape(DD, DD, SLICE)
    acp = np.ascontiguousarray(fc3.transpose(1, 0, 2))

    rf = np.asarray(rule_features)

    in_maps = []
    for c in range(N_CORES):
        k0 = KK * c
        c16_c = np.empty((128, S + KK), dtype=W_NP)
        c16_c[:, 0:S] = embedT
        c16_c[:, S : S + KK] = dise_sc[k0 : k0 + KK].T
        c32_c = np.empty((128, KK * REL + REL * S2), dtype=np.float32)
        c32_c[:, 0 : KK * REL] = B[k0 : k0 + KK].reshape(1, KK * REL)
        c32_c[:, KK * REL :] = binv_t
        rfT_c = np.ascontiguousarray(
            rf[k0 : k0 + KK].reshape(KK, RQ, 128).transpose(2, 0, 1).reshape(128, KK * RQ),
            dtype=W_NP,
        )

        w_c = np.empty((NDMA, 128, DMA_GROUP * 128), dtype=W_NP)
        wv = w_c.reshape(NDMA, 128, DMA_GROUP, 2, 64)  # [t2, k, i, sub, d]
        block = acp[k0 : k0 + KK]  # [kk, d, SLICE]
        for kk in range(KK):
            for s2 in range(S2):
                src = block[kk][:, R + s2 * 1024 : R + (s2 + 1) * 1024]
                chunkT = np.ascontiguousarray(
                    src.reshape(DD, 128, REL).transpose(2, 1, 0), dtype=W_NP
                )  # [r, p, d]
                for r in range(REL):
                    cc = kk * 256 + r * S2 + s2
                    t, sub = cc >> 1, cc & 1
                    wv[t >> 4, :, t & 15, sub, :] = chunkT[r]
            for q in range(RQ):
                cc = KK * 256 + kk * RQ + q
                t, sub = cc >> 1, cc & 1
                wv[t >> 4, :, t & 15, sub, :] = np.ascontiguousarray(
                    block[kk][:, q * 128 : (q + 1) * 128].T, dtype=W_NP
                )

        in_maps.append({"wt": w_c, "consts16": c16_c, "consts32": c32_c, "rfT": rfT_c})
    return in_maps


def finish(partials_list, fc_b):
    """Fold per-core [128, 2] psum outputs -> sigmoid(logits) [1, DD]."""
    acc = np.zeros(DD, dtype=np.float64)
    for p in partials_list:
        p = np.asarray(p, dtype=np.float64)
        acc += p[:DD, 0] + p[DD:, 1]
    logits = acc + np.asarray(fc_b, dtype=np.float64)
    out = 1.0 / (1.0 + np.exp(-logits))
    return out.reshape(1, DD).astype(np.float32)


def kernel(inputs, rule_features, embed_w, rel_w, dise_w, fc_w, fc_b):
    global LAST_RESULTS
    nc = get_nc()
    in_maps = make_in_maps(inputs, rule_features, embed_w, rel_w, dise_w, fc_w)
    LAST_RESULTS = run_bass_kernel_spmd(nc, in_maps, core_ids=list(range(N_CORES)))
    return finish([m["partials"] for m in LAST_RESULTS.results], fc_b)


# revision 12
# speedup vs baseline: 1.7589x; 1.7361x over previous
"""Trainium2 Bass kernel for nn_NeuroNet_231928234454 (retrieval_knn).

Computation (reference):
    mask = (inputs != 0)                                 [S]
    x    = (embed_w[:,None,:] + rel_w[None,:,:]) * mask  -> [S*REL, D]
    kg   = (dise_w @ x.T) / max(|dise_w| |x|, eps)       [DD, S*REL]  (cosine)
    hidden = concat([rule_features, kg], axis=1).reshape(1, -1)
    out  = sigmoid(hidden @ fc_w.T + fc_b)               [1, DD]

The 545 MB fc_w stream dominates (memory regime). Two key reductions:
  * mask sparsity: rows with inputs[s]==0 give kg==0 exactly, so their
    fc_w columns are dead — the host compacts to the ~S/2 live rows and
    never streams the dead weights (exact, not an approximation).
  * fp16 weight streaming (~1e-3 output rel err, HBM bytes halved).

Sharding: core c owns disease-rows 8c..8c+8 of the hidden matrix
[DD, 33280] (a column-shard of fc_w's input dim into 8 contiguous
blocks). Each core computes its 8 rows of kg on-chip from tiny inputs
(cosine identity: kg[k,(s,r)] = (A[k,s] + B[k,r]) * binv[s,r] with
A = (dise/|dise|) @ embed.T on the PE, B and binv host-precomputed),
then a PE mat-vec against its re-laid-out fc_w slice: each matmul's
stationary [128,128] packs two 128-element hidden chunks x 64 outputs,
rhs = the matching two hid columns, all accumulating into one PSUM
bank. Per-core [128,2] partials are folded + summed + sigmoid'ed on
host (64 numbers).
"""

import sys

if "/opt/trn_rl_repo" not in sys.path:
    sys.path.insert(0, "/opt/trn_rl_repo")

import numpy as np

import concourse.bacc as bacc
import concourse.mybir as mybir
from concourse.bass_utils import run_bass_kernel_spmd
from concourse.tile import TileContext

# -- problem constants (hardcoded; harness provides full-shape inputs) --
S, D, DD, REL, R = 4096, 128, 64, 8, 512
N_CORES = 8
KK = DD // N_CORES          # 8 disease rows per core
SLICE = REL * S + R         # 33280 hidden elems per disease row
DMA_GROUP = 16              # fc matmuls per W DMA tile
RQ = R // 128               # 4 chunks of rule features per disease

# fc_w streaming dtype. float32 is exact; float16 halves HBM traffic
# (the dominant cost) at ~1e-3 relative output error.
W_DTYPE = mybir.dt.float16
W_NP = np.float16
W_BUFS = 24 if W_NP == np.float16 else 14

LAST_RESULTS = None  # BassKernelResults of the most recent run (for test.py)
_CACHE = {}


def plan(s2v):
    """Derived sizes for a given number of live-s chunks (s2v = ceil(L/128))."""
    kg_cols = KK * REL * s2v
    nchunk = kg_cols + KK * RQ        # hid columns (128 elems each), even
    npair = nchunk // 2               # fc matmuls (2 chunks per stationary)
    assert npair % DMA_GROUP == 0
    ndma = npair // DMA_GROUP
    return kg_cols, nchunk, npair, ndma


def _build_bass(s2v):
    # Bacc (not raw Bass): its finalize() splits multi-sem waits into
    # event semaphores, which walrus codegen requires (max 1 wait/inst).
    nc = bacc.Bacc("TRN2", target_bir_lowering=False, debug=False)
    f32 = mybir.dt.float32
    wdt = W_DTYPE
    kg_cols, nchunk, npair, ndma = plan(s2v)
    sv = s2v * 128  # padded live-s count

    wt = nc.dram_tensor(
        "wt", [ndma, 128, DMA_GROUP * 128], wdt, kind="ExternalInput"
    )
    # Small constants packed into ONE tensor per dtype -> one DMA -> one
    # semaphore each (walrus allows only a single sync wait on a matmul's
    # LDWEIGHTS). fp16: [0:sv] embedT | [sv:sv+KK] diseT. f32:
    # [0:KK*REL] brep | [KK*REL:] binvT.
    C16_COLS = sv + KK
    C32_COLS = KK * REL + REL * s2v
    consts16 = nc.dram_tensor("consts16", [128, C16_COLS], wdt, kind="ExternalInput")
    consts32 = nc.dram_tensor("consts32", [128, C32_COLS], f32, kind="ExternalInput")
    rfT = nc.dram_tensor("rfT", [128, KK * RQ], wdt, kind="ExternalInput")
    partials = nc.dram_tensor("partials", [128, 2], f32, kind="ExternalOutput")

    with TileContext(nc) as tc:
        with (
            tc.tile_pool(name="const", bufs=1) as cpool,
            tc.tile_pool(name="wpool", bufs=W_BUFS) as wpool,
            tc.tile_pool(name="psum", bufs=1, space="PSUM") as ppool,
        ):
            c16_sb = cpool.tile([128, C16_COLS], wdt)
            nc.sync.dma_start(out=c16_sb[:], in_=consts16[:])
            c32_sb = cpool.tile([128, C32_COLS], f32)
            nc.sync.dma_start(out=c32_sb[:], in_=consts32[:])
            emb_sb = c16_sb[:, 0:sv]
            dise_sb = c16_sb[:, sv : sv + KK]
            brep_sb = c32_sb[:, 0 : KK * REL]
            binv_sb = c32_sb[:, KK * REL : C32_COLS]

            # hidden vector, chunk-column layout: hid[p, c] = hidden chunk c
            # elem p. c < kg_cols: (kk, r, s2) = kg for live row s2*128+p.
            # c >= kg_cols: (kk, q) = rule_features[kk, q*128+p].
            hid = cpool.tile([128, nchunk], wdt)
            nc.sync.dma_start(out=hid[:, kg_cols:nchunk], in_=rfT[:])

            # A[p, kk, s2] = sum_e embedT[e, s2*128+p] * diseT[e, kk]
            psumA = ppool.tile([128, KK, s2v], f32)
            for s2 in range(s2v):
                nc.tensor.matmul(
                    out=psumA[:, :, s2],
                    lhsT=emb_sb[:, s2 * 128 : (s2 + 1) * 128],
                    rhs=dise_sb,
                    start=True,
                    stop=True,
                )

            # kg -> hid: (A + B[kk,r]) * binv
            for kk in range(KK):
                for r in range(REL):
                    nc.vector.scalar_tensor_tensor(
                        out=hid[
                            :,
                            kk * REL * s2v + r * s2v : kk * REL * s2v + (r + 1) * s2v,
                        ],
                        in0=psumA[:, kk, :],
                        scalar=brep_sb[:, kk * REL + r : kk * REL + r + 1],
                        in1=binv_sb[:, r * s2v : (r + 1) * s2v],
                        op0=mybir.AluOpType.add,
                        op1=mybir.AluOpType.mult,
                    )

            # fc mat-vec: each stationary packs chunks (2t, 2t+1) as
            # [128, sub*64+d]; rhs = hid[:, 2t:2t+2]. psumF[d,0] and
            # psumF[64+d,1] accumulate the two true partial dots.
            psumF = ppool.tile([128, 2], f32)
            for t2 in range(ndma):
                wtile = wpool.tile([128, DMA_GROUP * 128], wdt)
                nc.sync.dma_start(out=wtile[:], in_=wt[t2, :, :])
                for i in range(DMA_GROUP):
                    t = t2 * DMA_GROUP + i
                    nc.tensor.matmul(
                        out=psumF[:, :],
                        lhsT=wtile[:, i * 128 : (i + 1) * 128],
                        rhs=hid[:, 2 * t : 2 * t + 2],
                        start=(t == 0),
                        stop=(t == npair - 1),
                    )

            out_sb = cpool.tile([128, 2], f32)
            nc.vector.tensor_copy(out=out_sb[:], in_=psumF[:, :])
            nc.sync.dma_start(out=partials[:], in_=out_sb[:])

    # Run Bacc's compile passes (register allocation, wait splitting into
    # event semaphores) — nothing else in the axon/PJRT path does this.
    nc.finalize()
    return nc


def get_nc(s2v):
    if s2v not in _CACHE:
        _CACHE[s2v] = _build_bass(s2v)
    return _CACHE[s2v]


def make_in_maps(inputs, rule_features, embed_w, rel_w, dise_w, fc_w):
    """Host-side sharding + relayout of the full inputs into per-core maps."""
    s_live = np.flatnonzero(np.asarray(inputs) != 0)  # live symptom rows
    ls = len(s_live)
    s2v = max(1, -(-ls // 128))  # ceil; >=1 keeps the program shape valid
    sv = s2v * 128
    kg_cols, nchunk, npair, ndma = plan(s2v)

    embed64 = np.asarray(embed_w, dtype=np.float64)
    rel64 = np.asarray(rel_w, dtype=np.float64)
    dise64 = np.asarray(dise_w, dtype=np.float64)
    el64 = embed64[s_live]  # [ls, D]

    # binv[s', r] = 1 / ||embed_s + rel_r|| for live rows (mask==1 there);
    # padded rows get 0 so their kg is exactly 0.
    sq = (
        (el64**2).sum(1)[:, None]
        + 2.0 * (el64 @ rel64.T)
        + (rel64**2).sum(1)[None, :]
    )  # [ls, REL]
    binv = np.zeros((sv, REL), dtype=np.float64)
    binv[:ls] = 1.0 / np.sqrt(np.maximum(sq, 1e-30))
    # [p, r*s2v + s2] with s' = s2*128+p
    binv_t = np.ascontiguousarray(
        binv.reshape(s2v, 128, REL).transpose(1, 2, 0).reshape(128, REL * s2v),
        dtype=np.float32,
    )

    dn = np.sqrt((dise64**2).sum(1))  # [DD]
    dise_sc = dise64 / dn[:, None]  # [DD, D]
    B = dise_sc @ rel64.T  # [DD, REL]

    # embedT for live rows, zero-padded: [128(e=D), sv]
    embedT = np.zeros((128, sv), dtype=W_NP)
    embedT[:, :ls] = np.asarray(embed_w)[s_live].T

    # Compact + transpose fc_w's kg part once for all cores:
    # acp_kg[kslot, d, s', r] = fc_w[d, kslot*SLICE + R + (s_live[s']*REL + r)]
    fc = np.asarray(fc_w)
    fcg = fc.reshape(DD, DD, SLICE)[:, :, R:].reshape(DD, DD, S, REL)
    acp_kg = np.ascontiguousarray(fcg[:, :, s_live, :].transpose(1, 0, 2, 3))
    # rf part: acp_rf[kslot, d, j] = fc_w[d, kslot*SLICE + j], j < R
    acp_rf = np.ascontiguousarray(
        fc.reshape(DD, DD, SLICE)[:, :, :R].transpose(1, 0, 2)
    )

    rf = np.asarray(rule_features)

    in_maps = []
    for c in range(N_CORES):
        k0 = KK * c
        c16_c = np.empty((128, sv + KK), dtype=W_NP)
        c16_c[:, 0:sv] = embedT
        c16_c[:, sv : sv + KK] = dise_sc[k0 : k0 + KK].T
        c32_c = np.empty((128, KK * REL + REL * s2v), dtype=np.float32)
        c32_c[:, 0 : KK * REL] = B[k0 : k0 + KK].reshape(1, KK * REL)
        c32_c[:, KK * REL :] = binv_t
        rfT_c = np.ascontiguousarray(
            rf[k0 : k0 + KK].reshape(KK, RQ, 128).transpose(2, 0, 1).reshape(128, KK * RQ),
            dtype=W_NP,
        )

        w_c = np.zeros((ndma, 128, DMA_GROUP * 128), dtype=W_NP)
        wv = w_c.reshape(ndma, 128, DMA_GROUP, 2, 64)  # [t2, k, i, sub, d]
        chunkT = np.zeros((REL, 128, DD), dtype=W_NP)
        for kk in range(KK):
            blk = acp_kg[k0 + kk]  # [d, ls, REL]
            for s2 in range(s2v):
                lo, hi = s2 * 128, min((s2 + 1) * 128, ls)
                if hi > lo:
                    chunkT[:, : hi - lo, :] = blk[:, lo:hi, :].transpose(2, 1, 0)
                    chunkT[:, hi - lo :, :] = 0
                else:
                    chunkT[:] = 0
                for r in range(REL):
                    cc = kk * REL * s2v + r * s2v + s2
                    t, sub = cc >> 1, cc & 1
                    wv[t // DMA_GROUP, :, t % DMA_GROUP, sub, :] = chunkT[r]
            for q in range(RQ):
                cc = kg_cols + kk * RQ + q
                t, sub = cc >> 1, cc & 1
                wv[t // DMA_GROUP, :, t % DMA_GROUP, sub, :] = np.ascontiguousarray(
                    acp_rf[k0 + kk][:, q * 128 : (q + 1) * 128].T, dtype=W_NP
                )

        in_maps.append({"wt": w_c, "consts16": c16_c, "consts32": c32_c, "rfT": rfT_c})
    return s2v, in_maps


def finish(partials_list, fc_b):
    """Fold per-core [128, 2] psum outputs -> sigmoid(logits) [1, DD]."""
    acc = np.zeros(DD, dtype=np.float64)
    for p in partials_list:
        p = np.asarray(p, dtype=np.float64)
        acc += p[:DD, 0] + p[DD:, 1]
    logits = acc + np.asarray(fc_b, dtype=np.float64)
    out = 1.0 / (1.0 + np.exp(-logits))
    return out.reshape(1, DD).astype(np.float32)


def kernel(inputs, rule_features, embed_w, rel_w, dise_w, fc_w, fc_b):
    global LAST_RESULTS
    s2v, in_maps = make_in_maps(
        inputs, rule_features, embed_w, rel_w, dise_w, fc_w
    )
    nc = get_nc(s2v)
    LAST_RESULTS = run_bass_kernel_spmd(nc, in_maps, core_ids=list(range(N_CORES)))
    return finish([m["partials"] for m in LAST_RESULTS.results], fc_b)
